# revision 6
# baseline (speedup 1.0000x reference)
"""GroupedQueryAttention kernel for 8 Trainium2 NeuronCores.

Strategy:
  - Head-sharded tensor parallelism: core c owns Q heads {2c, 2c+1} and
    their KV head c//2 (16 Q heads / 4 KV heads, head_dim 64).
  - Phase 1 (per core): stream-transpose x, project Q^T/K^T/V^T with
    fp32r matmuls, RoPE, then flash attention in transposed (S^T) layout
    so P^T feeds the PV matmul directly. Softmax denominator comes from a
    ones-column appended to V. Document masking is handled by compiling
    the tile loop for the doc segment boundaries (data-dependent JIT);
    causal masking inside diagonal tiles uses 4 static offset masks.
  - Host reshards attention output (pure concat/slice), phase 2 computes
    the output projection sequence-sharded across cores.
"""

import sys

for _p in ("/opt/trn_rl_repo",):
    if _p not in sys.path:
        sys.path.append(_p)

import functools

import numpy as np

import concourse.bass as bass
import concourse.mybir as mybir
from concourse import bacc
from concourse.bass_utils import run_bass_kernel_spmd
from concourse.masks import make_identity
from concourse.tile import TileContext

F32 = mybir.dt.float32
F32R = mybir.dt.float32r
AF = mybir.ActivationFunctionType

N_CORES = 8
T = 4096
D = 1024
NH = 16
NKV = 4
DH = 64
QC = 512    # q chunk (free dim of S^T tiles)
KC = 128    # k tile (partition dim of S^T tiles)
TC = 512    # token chunk in projection stage
SCALE = 1.0 / 8.0


def _segments(doc_ids):
    """Contiguous runs of equal doc id: list of (start, end, value)."""
    d = np.asarray(doc_ids).reshape(-1)
    bounds = [0] + (np.nonzero(d[1:] != d[:-1])[0] + 1).tolist() + [len(d)]
    return tuple(
        (int(bounds[i]), int(bounds[i + 1]), int(d[bounds[i]]))
        for i in range(len(bounds) - 1)
    )


def _attn_blocks(segs):
    """For each q-run: the k-ranges it may attend to.

    Returns list of (qs, qe, [(ks, ke, is_self)]), where is_self marks the
    causal-triangle block (same run). Non-self blocks are fully visible
    (entirely earlier same-value runs)."""
    out = []
    for qi, (qs, qe, qv) in enumerate(segs):
        blocks = []
        for ki in range(qi):
            ks, ke, kv = segs[ki]
            if kv == qv:
                blocks.append((ks, ke, False))
        blocks.append((qs, qe, True))
        out.append((qs, qe, blocks))
    return out


@functools.lru_cache(maxsize=8)
def _build_phase1(segs):
    nc = bacc.Bacc("TRN2", target_bir_lowering=False, debug=False,
                   num_devices=N_CORES)

    xin = nc.declare_dram_parameter("xin", [T, D], F32, isOutput=False)
    sc = nc.declare_dram_parameter("sc", [T, 2 * DH], F32, isOutput=False)
    wq = nc.declare_dram_parameter("wq", [D, 2 * DH], F32, isOutput=False)
    wkv = nc.declare_dram_parameter("wkv", [D, 2 * DH], F32, isOutput=False)
    ot = nc.declare_dram_parameter("ot", [2 * DH, T], F32, isOutput=True)

    blocks = _attn_blocks(segs)
    # doc-aligned V tile grid: (k0, klen, vaug tile idx)
    vtiles = []
    vidx_of = {}
    for qs, qe, _v in segs:
        for k0 in range(qs, qe, KC):
            vidx_of[k0] = len(vtiles)
            vtiles.append((k0, min(KC, qe - k0)))
    nvt = len(vtiles)

    with TileContext(nc) as tc:
        with (
            tc.tile_pool(name="const", bufs=1) as const,
            tc.tile_pool(name="big", bufs=1) as big,
        ):
            ident = const.tile([128, 128], F32)
            make_identity(nc, ident)
            masks = []
            for mi in range(4):
                mf = const.tile([KC, QC], F32, tag=f"maskf{mi}")
                nc.any.memset(mf[:, :], 1.0)
                # keep where q - k - 128*mi >= 0
                nc.gpsimd.affine_select(
                    out=mf[:, :], in_=mf[:, :],
                    compare_op=mybir.AluOpType.is_ge, fill=0.0,
                    base=-(128 * mi), pattern=[[1, QC]], channel_multiplier=-1,
                )
                m = const.tile([KC, QC], F32R, tag=f"mask{mi}")
                nc.vector.tensor_copy(m[:, :], mf[:, :])
                masks.append(m)
            onecol = const.tile([128, 1], F32, tag="onecol")
            nc.any.memset(onecol[:, :], 1.0)
            zpad = const.tile([DH, 8], F32, tag="zpad")
            nc.any.memset(zpad[:, :], 0.0)

            wq_sb = big.tile([128, 8, 128], F32R, tag="wq")
            wkv_sb = big.tile([128, 8, 128], F32R, tag="wkv")
            nc.gpsimd.dma_start(
                out=wq_sb[:, :, :],
                in_=wq.rearrange("(c p) n -> p c n", p=128))
            nc.gpsimd.dma_start(
                out=wkv_sb[:, :, :],
                in_=wkv.rearrange("(c p) n -> p c n", p=128))

            sincosT = big.tile([128, T], F32, tag="sincosT")
            qT = big.tile([DH, 2, T + 8], F32R, tag="qT")
            kT = big.tile([DH, T], F32R, tag="kT")
            vT = big.tile([DH, T], F32, tag="vT")
            vaug = big.tile([128, nvt, DH + 1], F32R, tag="vaug")
            for h in range(2):
                nc.vector.tensor_copy(qT[:, h, T:T + 8], zpad[:, :])

            # ---- stage B: transpose x + project qkv + rope ----
            with (
                tc.tile_pool(name="xld", bufs=4) as xld,
                tc.tile_pool(name="xTp", bufs=4) as xTp,
                tc.tile_pool(name="rtmp", bufs=3) as rtmp,
                tc.tile_pool(name="psT", bufs=2, space="PSUM") as psT,
                tc.tile_pool(name="psQKV", bufs=2, space="PSUM") as psQKV,
            ):
                # sin|cos transpose: sincosT rows 0:64 sin^T, 64:128 cos^T
                for t in range(8):
                    sld = xld.tile([128, 4, 128], F32, tag="x")
                    nc.sync.dma_start(
                        out=sld[:, :, :],
                        in_=sc[t * TC:(t + 1) * TC, :].rearrange(
                            "(s p) f -> p s f", p=128))
                    pT = psT.tile([128, TC], F32, tag="t")
                    for s in range(4):
                        nc.tensor.transpose(
                            pT[:, s * 128:(s + 1) * 128],
                            sld[:, s, :], ident[:, :])
                    nc.vector.tensor_copy(
                        sincosT[:, t * TC:(t + 1) * TC], pT[:, :])

                for t in range(8):
                    pQKV = psQKV.tile([128, 1024], F32, tag="qkv")
                    for d in range(8):
                        xt = xld.tile([128, 4, 128], F32, tag="x")
                        nc.sync.dma_start(
                            out=xt[:, :, :],
                            in_=xin[t * TC:(t + 1) * TC,
                                    d * 128:(d + 1) * 128].rearrange(
                                        "(s p) f -> p s f", p=128))
                        pT = psT.tile([128, TC], F32, tag="t")
                        for s in range(4):
                            nc.tensor.transpose(
                                pT[:, s * 128:(s + 1) * 128],
                                xt[:, s, :], ident[:, :])
                        xT = xTp.tile([128, TC], F32R, tag="xT")
                        if d % 2 == 0:
                            nc.vector.tensor_copy(xT[:, :], pT[:, :])
                        else:
                            nc.scalar.copy(xT[:, :], pT[:, :])
                        nc.tensor.matmul(
                            pQKV[:, 0:TC], wq_sb[:, d, :], xT[:, :],
                            start=(d == 0), stop=(d == 7))
                        nc.tensor.matmul(
                            pQKV[:, TC:2 * TC], wkv_sb[:, d, :], xT[:, :],
                            start=(d == 0), stop=(d == 7))

                    cosq = sincosT[64:128, t * TC:(t + 1) * TC]
                    sinq = sincosT[0:64, t * TC:(t + 1) * TC]
                    tsl = (t * TC, (t + 1) * TC)

                    def rope(dst, src_base, col0):
                        # src rows [src_base, src_base+64) of pQKV cols
                        # [col0, col0+512); dst [64, 512] slice of SBUF
                        rot = rtmp.tile([DH, TC], F32, tag="rot")
                        cs = rtmp.tile([DH, TC], F32, tag="cs")
                        nc.vector.tensor_scalar_mul(
                            rot[0:32, :],
                            pQKV[src_base + 32:src_base + 64, col0:col0 + TC],
                            -1.0)
                        nc.vector.tensor_copy(
                            rot[32:64, :],
                            pQKV[src_base:src_base + 32, col0:col0 + TC])
                        nc.vector.tensor_mul(
                            cs[:, :],
                            pQKV[src_base:src_base + 64, col0:col0 + TC],
                            cosq)
                        nc.vector.tensor_mul(rot[:, :], rot[:, :], sinq)
                        nc.vector.tensor_add(dst, cs[:, :], rot[:, :])

                    rope(qT[:, 0, tsl[0]:tsl[1]], 0, 0)
                    rope(qT[:, 1, tsl[0]:tsl[1]], 64, 0)
                    rope(kT[:, tsl[0]:tsl[1]], 0, TC)
                    nc.scalar.copy(vT[:, tsl[0]:tsl[1]],
                                   pQKV[64:128, TC:2 * TC])

                # vaug: doc-aligned token-major V tiles (+ ones col)
                with tc.tile_pool(name="psV", bufs=2, space="PSUM") as psV:
                    for (k0, klen) in vtiles:
                        pv = psV.tile([128, DH], F32, tag="v")
                        nc.tensor.transpose(
                            pv[0:klen, 0:DH],
                            vT[:, k0:k0 + klen],
                            ident[0:DH, 0:DH])
                        nc.any.tensor_copy(
                            vaug[0:klen, vidx_of[k0], 0:DH],
                            pv[0:klen, :])
                        nc.vector.tensor_copy(
                            vaug[0:klen, vidx_of[k0], DH:DH + 1],
                            onecol[0:klen, :])

            # ---- stage C: attention ----
            with (
                tc.tile_pool(name="psS", bufs=4, space="PSUM") as psS,
                tc.tile_pool(name="psO", bufs=2, space="PSUM") as psO,
                tc.tile_pool(name="pp", bufs=6) as pp,
                tc.tile_pool(name="npool", bufs=2) as npool,
            ):
                for h in range(2):
                    for qs, qe, blks in blocks:
                        for q0 in range(qs, qe, QC):
                            qlen = min(QC, qe - q0)
                            qpad = qlen + (qlen & 1)
                            # gather k tiles for this q chunk
                            ktiles = []
                            for ks, ke, is_self in blks:
                                kend = ke if not is_self else min(
                                    q0 + qlen, ke)
                                for k0 in range(ks, kend, KC):
                                    klen = min(KC, kend - k0)
                                    # causal mask needed?
                                    dlt = k0 - q0
                                    need = is_self and (k0 + klen - 1 > q0)
                                    ktiles.append((k0, klen, need, dlt))
                            po = psO.tile([DH + 1, QC], F32, tag="o")
                            nk = len(ktiles)
                            for ki, (k0, klen, need, dlt) in enumerate(ktiles):
                                ps = psS.tile([KC, QC], F32, tag="s")
                                nc.tensor.matmul(
                                    ps[0:klen, 0:qpad],
                                    kT[:, k0:k0 + klen],
                                    qT[:, h, q0:q0 + qpad],
                                    start=True, stop=True)
                                pt = pp.tile([KC, QC], F32R, tag="p")
                                nc.scalar.activation(
                                    pt[0:klen, 0:qpad], ps[0:klen, 0:qpad],
                                    AF.Exp, scale=SCALE)
                                if need:
                                    assert dlt % 128 == 0 and 0 <= dlt < 512, dlt
                                    nc.vector.tensor_mul(
                                        pt[0:klen, 0:qpad],
                                        pt[0:klen, 0:qpad],
                                        masks[dlt // 128][0:klen, 0:qpad])
                                nc.tensor.matmul(
                                    po[0:DH + 1, 0:qpad],
                                    vaug[0:klen, vidx_of[k0], :],
                                    pt[0:klen, 0:qpad],
                                    start=(ki == 0), stop=(ki == nk - 1))
                            rc = npool.tile([1, QC], F32, tag="rc")
                            nc.vector.reciprocal(
                                rc[0:1, 0:qlen], po[DH:DH + 1, 0:qlen])
                            rb = npool.tile([DH, QC], F32, tag="rb")
                            nc.gpsimd.partition_broadcast(
                                rb[:, 0:qlen], rc[0:1, 0:qlen])
                            ob = npool.tile([DH, QC], F32, tag="ob")
                            nc.vector.tensor_mul(
                                ob[:, 0:qlen], po[0:DH, 0:qlen], rb[:, 0:qlen])
                            nc.sync.dma_start(
                                out=ot[h * DH:(h + 1) * DH, q0:q0 + qlen],
                                in_=ob[:, 0:qlen])

    nc.compile()
    return nc


@functools.lru_cache(maxsize=1)
def _build_phase2():
    nc = bacc.Bacc("TRN2", target_bir_lowering=False, debug=False,
                   num_devices=N_CORES)
    TL = T // N_CORES  # 512 tokens per core
    at = nc.declare_dram_parameter("at", [D, TL], F32, isOutput=False)
    wo = nc.declare_dram_parameter("wo", [D, D], F32, isOutput=False)
    ot2 = nc.declare_dram_parameter("ot2", [D, TL], F32, isOutput=True)

    with TileContext(nc) as tc:
        with (
            tc.tile_pool(name="big", bufs=1) as big,
            tc.tile_pool(name="ps", bufs=4, space="PSUM") as ps,
            tc.tile_pool(name="ob", bufs=4) as obp,
        ):
            wo_sb = big.tile([128, 8, D], F32R, tag="wo")
            at_sb = big.tile([128, 8, TL], F32R, tag="at")
            nc.gpsimd.dma_start(
                out=wo_sb[:, :, :],
                in_=wo.rearrange("(c p) n -> p c n", p=128))
            nc.gpsimd.dma_start(
                out=at_sb[:, :, :],
                in_=at.rearrange("(c p) n -> p c n", p=128))
            for m in range(8):
                po = ps.tile([128, TL], F32, tag="o")
                for kc in range(8):
                    nc.tensor.matmul(
                        po[:, :],
                        wo_sb[:, kc, m * 128:(m + 1) * 128],
                        at_sb[:, kc, :],
                        start=(kc == 0), stop=(kc == 7))
                ob = obp.tile([128, TL], F32, tag="ob")
                nc.any.tensor_copy(ob[:, :], po[:, :])
                nc.sync.dma_start(
                    out=ot2[m * 128:(m + 1) * 128, :], in_=ob[:, :])

    nc.compile()
    return nc


def kernel(x, sin, cos, W_qkv, W_out, doc_ids):
    x = np.asarray(x, dtype=np.float32)
    sin = np.asarray(sin, dtype=np.float32)
    cos = np.asarray(cos, dtype=np.float32)
    W_qkv = np.asarray(W_qkv, dtype=np.float32)
    W_out = np.asarray(W_out, dtype=np.float32)

    x2 = np.ascontiguousarray(x.reshape(T, D))
    sc = np.ascontiguousarray(np.concatenate([sin, cos], axis=1))

    segs = _segments(doc_ids)
    nc1 = _build_phase1(segs)

    in_maps1 = []
    for c in range(N_CORES):
        g = c // 2
        wq_c = np.ascontiguousarray(W_qkv[:, 2 * c * DH:(2 * c + 2) * DH])
        wkv_c = np.ascontiguousarray(np.concatenate(
            [W_qkv[:, D + g * DH:D + (g + 1) * DH],
             W_qkv[:, D + NKV * DH + g * DH:D + NKV * DH + (g + 1) * DH]],
            axis=1))
        in_maps1.append({"xin": x2, "sc": sc, "wq": wq_c, "wkv": wkv_c})

    r1 = run_bass_kernel_spmd(nc1, in_maps1, list(range(N_CORES)))
    attn_t = np.concatenate(
        [r1.results[c]["ot"] for c in range(N_CORES)], axis=0)  # [1024, 4096]

    nc2 = _build_phase2()
    TL = T // N_CORES
    in_maps2 = [
        {"at": np.ascontiguousarray(attn_t[:, c * TL:(c + 1) * TL]),
         "wo": W_out}
        for c in range(N_CORES)
    ]
    r2 = run_bass_kernel_spmd(nc2, in_maps2, list(range(N_CORES)))
    out_t = np.concatenate(
        [r2.results[c]["ot2"] for c in range(N_CORES)], axis=1)  # [1024, 4096]
    return np.ascontiguousarray(out_t.T).reshape(1, T, D)


# revision 8
# speedup vs baseline: 1.0544x; 1.0544x over previous
"""GroupedQueryAttention kernel for 8 Trainium2 NeuronCores.

Strategy:
  - Head-sharded tensor parallelism: core c owns Q heads {2c, 2c+1} and
    their KV head c//2 (16 Q heads / 4 KV heads, head_dim 64).
  - Phase 1 (per core): stream-transpose x, project Q^T/K^T/V^T with
    fp32r matmuls, RoPE, then flash attention in transposed (S^T) layout
    so P^T feeds the PV matmul directly. Softmax denominator comes from a
    ones-column appended to V. Document masking is handled by compiling
    the tile loop for the doc segment boundaries (data-dependent JIT);
    causal masking inside diagonal tiles uses 4 static offset masks.
  - Host reshards attention output (pure concat/slice), phase 2 computes
    the output projection sequence-sharded across cores.
"""

import sys

for _p in ("/opt/trn_rl_repo",):
    if _p not in sys.path:
        sys.path.append(_p)

import functools

import numpy as np

import concourse.bass as bass
import concourse.mybir as mybir
from concourse import bacc
from concourse.bass_utils import run_bass_kernel_spmd
from concourse.masks import make_identity
from concourse.tile import TileContext

F32 = mybir.dt.float32
F32R = mybir.dt.float32r
AF = mybir.ActivationFunctionType

N_CORES = 8
T = 4096
D = 1024
NH = 16
NKV = 4
DH = 64
QC = 512    # q chunk (free dim of S^T tiles)
KC = 128    # k tile (partition dim of S^T tiles)
TC = 512    # token chunk in projection stage
SCALE = 1.0 / 8.0


def _segments(doc_ids):
    """Contiguous runs of equal doc id: list of (start, end, value)."""
    d = np.asarray(doc_ids).reshape(-1)
    bounds = [0] + (np.nonzero(d[1:] != d[:-1])[0] + 1).tolist() + [len(d)]
    return tuple(
        (int(bounds[i]), int(bounds[i + 1]), int(d[bounds[i]]))
        for i in range(len(bounds) - 1)
    )


def _attn_blocks(segs):
    """For each q-run: the k-ranges it may attend to.

    Returns list of (qs, qe, [(ks, ke, is_self)]), where is_self marks the
    causal-triangle block (same run). Non-self blocks are fully visible
    (entirely earlier same-value runs)."""
    out = []
    for qi, (qs, qe, qv) in enumerate(segs):
        blocks = []
        for ki in range(qi):
            ks, ke, kv = segs[ki]
            if kv == qv:
                blocks.append((ks, ke, False))
        blocks.append((qs, qe, True))
        out.append((qs, qe, blocks))
    return out


@functools.lru_cache(maxsize=8)
def _build_phase1(segs):
    nc = bacc.Bacc("TRN2", target_bir_lowering=False, debug=False,
                   num_devices=N_CORES)

    xin = nc.declare_dram_parameter("xin", [T, D], F32, isOutput=False)
    sc = nc.declare_dram_parameter("sc", [T, 2 * DH], F32, isOutput=False)
    wq = nc.declare_dram_parameter("wq", [D, 2 * DH], F32, isOutput=False)
    wkv = nc.declare_dram_parameter("wkv", [D, 2 * DH], F32, isOutput=False)
    ot = nc.declare_dram_parameter("ot", [2 * DH, T], F32, isOutput=True)

    blocks = _attn_blocks(segs)
    # doc-aligned V tile grid: (k0, klen, vaug tile idx)
    vtiles = []
    vidx_of = {}
    for qs, qe, _v in segs:
        for k0 in range(qs, qe, KC):
            vidx_of[k0] = len(vtiles)
            vtiles.append((k0, min(KC, qe - k0)))
    nvt = len(vtiles)

    with TileContext(nc) as tc:
        with (
            tc.tile_pool(name="const", bufs=1) as const,
            tc.tile_pool(name="big", bufs=1) as big,
        ):
            ident = const.tile([128, 128], F32)
            make_identity(nc, ident)
            masks = []
            for mi in range(4):
                mf = const.tile([KC, QC], F32, tag=f"maskf{mi}")
                nc.any.memset(mf[:, :], 1.0)
                # keep where q - k - 128*mi >= 0
                nc.gpsimd.affine_select(
                    out=mf[:, :], in_=mf[:, :],
                    compare_op=mybir.AluOpType.is_ge, fill=0.0,
                    base=-(128 * mi), pattern=[[1, QC]], channel_multiplier=-1,
                )
                m = const.tile([KC, QC], F32R, tag=f"mask{mi}")
                nc.vector.tensor_copy(m[:, :], mf[:, :])
                masks.append(m)
            onecol = const.tile([128, 1], F32, tag="onecol")
            nc.any.memset(onecol[:, :], 1.0)
            zpad = const.tile([DH, 8], F32, tag="zpad")
            nc.any.memset(zpad[:, :], 0.0)

            wq_sb = big.tile([128, 8, 128], F32R, tag="wq")
            wkv_sb = big.tile([128, 8, 128], F32R, tag="wkv")
            nc.gpsimd.dma_start(
                out=wq_sb[:, :, :],
                in_=wq.rearrange("(c p) n -> p c n", p=128))
            nc.gpsimd.dma_start(
                out=wkv_sb[:, :, :],
                in_=wkv.rearrange("(c p) n -> p c n", p=128))

            sincosT = big.tile([128, T], F32, tag="sincosT")
            qT = big.tile([DH, 2, T + 8], F32R, tag="qT")
            kT = big.tile([DH, T], F32R, tag="kT")
            vT = big.tile([DH, T], F32, tag="vT")
            vaug = big.tile([128, nvt, DH + 1], F32R, tag="vaug")
            for h in range(2):
                nc.vector.tensor_copy(qT[:, h, T:T + 8], zpad[:, :])

            # ---- stage B: transpose x + project qkv + rope ----
            with (
                tc.tile_pool(name="xld", bufs=4) as xld,
                tc.tile_pool(name="xTp", bufs=10) as xTp,
                tc.tile_pool(name="rtmp", bufs=3) as rtmp,
                tc.tile_pool(name="psT", bufs=2, space="PSUM") as psT,
                tc.tile_pool(name="psQKV", bufs=2, space="PSUM") as psQKV,
            ):
                # sin|cos transpose: sincosT rows 0:64 sin^T, 64:128 cos^T
                for t in range(8):
                    sld = xld.tile([128, 4, 128], F32, tag="x")
                    nc.sync.dma_start(
                        out=sld[:, :, :],
                        in_=sc[t * TC:(t + 1) * TC, :].rearrange(
                            "(s p) f -> p s f", p=128))
                    pT = psT.tile([128, TC], F32, tag="t")
                    for s in range(4):
                        nc.tensor.transpose(
                            pT[:, s * 128:(s + 1) * 128],
                            sld[:, s, :], ident[:, :])
                    nc.vector.tensor_copy(
                        sincosT[:, t * TC:(t + 1) * TC], pT[:, :])

                for t in range(8):
                    pQKV = psQKV.tile([128, 1024], F32, tag="qkv")
                    xTs = []
                    for d in range(8):
                        xt = xld.tile([128, 4, 128], F32, tag="x")
                        nc.sync.dma_start(
                            out=xt[:, :, :],
                            in_=xin[t * TC:(t + 1) * TC,
                                    d * 128:(d + 1) * 128].rearrange(
                                        "(s p) f -> p s f", p=128))
                        pT = psT.tile([128, TC], F32, tag="t")
                        for s in range(4):
                            nc.tensor.transpose(
                                pT[:, s * 128:(s + 1) * 128],
                                xt[:, s, :], ident[:, :])
                        xT = xTp.tile([128, TC], F32R, tag="xT")
                        if d % 2 == 0:
                            nc.vector.tensor_copy(xT[:, :], pT[:, :])
                        else:
                            nc.scalar.copy(xT[:, :], pT[:, :])
                        xTs.append(xT)
                    for d in range(8):
                        nc.tensor.matmul(
                            pQKV[:, 0:TC], wq_sb[:, d, :], xTs[d][:, :],
                            start=(d == 0), stop=(d == 7))
                        nc.tensor.matmul(
                            pQKV[:, TC:2 * TC], wkv_sb[:, d, :], xTs[d][:, :],
                            start=(d == 0), stop=(d == 7))

                    cosq = sincosT[64:128, t * TC:(t + 1) * TC]
                    sinq = sincosT[0:64, t * TC:(t + 1) * TC]
                    tsl = (t * TC, (t + 1) * TC)

                    def rope(dst, src_base, col0):
                        # src rows [src_base, src_base+64) of pQKV cols
                        # [col0, col0+512); dst [64, 512] slice of SBUF
                        rot = rtmp.tile([DH, TC], F32, tag="rot")
                        cs = rtmp.tile([DH, TC], F32, tag="cs")
                        nc.vector.tensor_scalar_mul(
                            rot[0:32, :],
                            pQKV[src_base + 32:src_base + 64, col0:col0 + TC],
                            -1.0)
                        nc.vector.tensor_copy(
                            rot[32:64, :],
                            pQKV[src_base:src_base + 32, col0:col0 + TC])
                        nc.vector.tensor_mul(
                            cs[:, :],
                            pQKV[src_base:src_base + 64, col0:col0 + TC],
                            cosq)
                        nc.vector.tensor_mul(rot[:, :], rot[:, :], sinq)
                        nc.vector.tensor_add(dst, cs[:, :], rot[:, :])

                    rope(qT[:, 0, tsl[0]:tsl[1]], 0, 0)
                    rope(qT[:, 1, tsl[0]:tsl[1]], 64, 0)
                    rope(kT[:, tsl[0]:tsl[1]], 0, TC)
                    nc.scalar.copy(vT[:, tsl[0]:tsl[1]],
                                   pQKV[64:128, TC:2 * TC])

                # vaug: doc-aligned token-major V tiles (+ ones col)
                with tc.tile_pool(name="psV", bufs=2, space="PSUM") as psV:
                    for (k0, klen) in vtiles:
                        pv = psV.tile([128, DH], F32, tag="v")
                        nc.tensor.transpose(
                            pv[0:klen, 0:DH],
                            vT[:, k0:k0 + klen],
                            ident[0:DH, 0:DH])
                        nc.any.tensor_copy(
                            vaug[0:klen, vidx_of[k0], 0:DH],
                            pv[0:klen, :])
                        nc.vector.tensor_copy(
                            vaug[0:klen, vidx_of[k0], DH:DH + 1],
                            onecol[0:klen, :])

            # ---- stage C: attention ----
            with (
                tc.tile_pool(name="psS", bufs=4, space="PSUM") as psS,
                tc.tile_pool(name="psO", bufs=4, space="PSUM") as psO,
                tc.tile_pool(name="pp", bufs=6) as pp,
                tc.tile_pool(name="npool", bufs=2) as npool,
            ):
                for h in range(2):
                    for qs, qe, blks in blocks:
                        for q0 in range(qs, qe, QC):
                            qlen = min(QC, qe - q0)
                            qpad = qlen + (qlen & 1)
                            # gather k tiles for this q chunk
                            ktiles = []
                            for ks, ke, is_self in blks:
                                kend = ke if not is_self else min(
                                    q0 + qlen, ke)
                                for k0 in range(ks, kend, KC):
                                    klen = min(KC, kend - k0)
                                    # causal mask needed?
                                    dlt = k0 - q0
                                    need = is_self and (k0 + klen - 1 > q0)
                                    ktiles.append((k0, klen, need, dlt))
                            po = psO.tile([DH + 1, QC], F32, tag="o")
                            nk = len(ktiles)
                            for ki, (k0, klen, need, dlt) in enumerate(ktiles):
                                ps = psS.tile([KC, QC], F32, tag="s")
                                nc.tensor.matmul(
                                    ps[0:klen, 0:qpad],
                                    kT[:, k0:k0 + klen],
                                    qT[:, h, q0:q0 + qpad],
                                    start=True, stop=True)
                                pt = pp.tile([KC, QC], F32R, tag="p")
                                nc.scalar.activation(
                                    pt[0:klen, 0:qpad], ps[0:klen, 0:qpad],
                                    AF.Exp, scale=SCALE)
                                if need:
                                    assert dlt % 128 == 0 and 0 <= dlt < 512, dlt
                                    nc.vector.tensor_mul(
                                        pt[0:klen, 0:qpad],
                                        pt[0:klen, 0:qpad],
                                        masks[dlt // 128][0:klen, 0:qpad])
                                nc.tensor.matmul(
                                    po[0:DH + 1, 0:qpad],
                                    vaug[0:klen, vidx_of[k0], :],
                                    pt[0:klen, 0:qpad],
                                    start=(ki == 0), stop=(ki == nk - 1))
                            rc = npool.tile([1, QC], F32, tag="rc")
                            nc.vector.reciprocal(
                                rc[0:1, 0:qlen], po[DH:DH + 1, 0:qlen])
                            rb = npool.tile([DH, QC], F32, tag="rb")
                            nc.gpsimd.partition_broadcast(
                                rb[:, 0:qlen], rc[0:1, 0:qlen])
                            ob = npool.tile([DH, QC], F32, tag="ob")
                            nc.vector.tensor_mul(
                                ob[:, 0:qlen], po[0:DH, 0:qlen], rb[:, 0:qlen])
                            nc.sync.dma_start(
                                out=ot[h * DH:(h + 1) * DH, q0:q0 + qlen],
                                in_=ob[:, 0:qlen])

    nc.compile()
    return nc


@functools.lru_cache(maxsize=1)
def _build_phase2():
    nc = bacc.Bacc("TRN2", target_bir_lowering=False, debug=False,
                   num_devices=N_CORES)
    TL = T // N_CORES  # 512 tokens per core
    at = nc.declare_dram_parameter("at", [D, TL], F32, isOutput=False)
    wo = nc.declare_dram_parameter("wo", [D, D], F32, isOutput=False)
    ot2 = nc.declare_dram_parameter("ot2", [D, TL], F32, isOutput=True)

    with TileContext(nc) as tc:
        with (
            tc.tile_pool(name="big", bufs=1) as big,
            tc.tile_pool(name="ps", bufs=4, space="PSUM") as ps,
            tc.tile_pool(name="ob", bufs=4) as obp,
        ):
            wo_sb = big.tile([128, 8, D], F32R, tag="wo")
            at_sb = big.tile([128, 8, TL], F32R, tag="at")
            for kc in range(8):
                nc.gpsimd.dma_start(
                    out=at_sb[:, kc, :],
                    in_=at[kc * 128:(kc + 1) * 128, :])
                nc.gpsimd.dma_start(
                    out=wo_sb[:, kc, :],
                    in_=wo[kc * 128:(kc + 1) * 128, :])
            for m in range(8):
                po = ps.tile([128, TL], F32, tag="o")
                for kc in range(8):
                    nc.tensor.matmul(
                        po[:, :],
                        wo_sb[:, kc, m * 128:(m + 1) * 128],
                        at_sb[:, kc, :],
                        start=(kc == 0), stop=(kc == 7))
                ob = obp.tile([128, TL], F32, tag="ob")
                nc.any.tensor_copy(ob[:, :], po[:, :])
                nc.sync.dma_start(
                    out=ot2[m * 128:(m + 1) * 128, :], in_=ob[:, :])

    nc.compile()
    return nc


def kernel(x, sin, cos, W_qkv, W_out, doc_ids):
    x = np.asarray(x, dtype=np.float32)
    sin = np.asarray(sin, dtype=np.float32)
    cos = np.asarray(cos, dtype=np.float32)
    W_qkv = np.asarray(W_qkv, dtype=np.float32)
    W_out = np.asarray(W_out, dtype=np.float32)

    x2 = np.ascontiguousarray(x.reshape(T, D))
    sc = np.ascontiguousarray(np.concatenate([sin, cos], axis=1))

    segs = _segments(doc_ids)
    nc1 = _build_phase1(segs)

    in_maps1 = []
    for c in range(N_CORES):
        g = c // 2
        wq_c = np.ascontiguousarray(W_qkv[:, 2 * c * DH:(2 * c + 2) * DH])
        wkv_c = np.ascontiguousarray(np.concatenate(
            [W_qkv[:, D + g * DH:D + (g + 1) * DH],
             W_qkv[:, D + NKV * DH + g * DH:D + NKV * DH + (g + 1) * DH]],
            axis=1))
        in_maps1.append({"xin": x2, "sc": sc, "wq": wq_c, "wkv": wkv_c})

    r1 = run_bass_kernel_spmd(nc1, in_maps1, list(range(N_CORES)))
    attn_t = np.concatenate(
        [r1.results[c]["ot"] for c in range(N_CORES)], axis=0)  # [1024, 4096]

    nc2 = _build_phase2()
    TL = T // N_CORES
    in_maps2 = [
        {"at": np.ascontiguousarray(attn_t[:, c * TL:(c + 1) * TL]),
         "wo": W_out}
        for c in range(N_CORES)
    ]
    r2 = run_bass_kernel_spmd(nc2, in_maps2, list(range(N_CORES)))
    out_t = np.concatenate(
        [r2.results[c]["ot2"] for c in range(N_CORES)], axis=1)  # [1024, 4096]
    return np.ascontiguousarray(out_t.T).reshape(1, T, D)
